# revision 2
# baseline (speedup 1.0000x reference)
"""Depthwise Conv1d (C=512, K=3, stride=1, pad=1) on 8 Trainium2 NeuronCores.

Problem: x [16, 512, 4096] f32, w [512, 1, 3] f32, b [512] f32
         out[n,c,l] = sum_k w[c,0,k] * x_pad[n,c,l+k] + b[c]

Sharding: data-parallel over batch — 2 batches per core; each core handles
all 512 channels as 4 blocks of 128 partitions, each row split in two
halo-overlapped halves -> 16 work tiles of [128, 2048] per core.

The kernel computes in bf16 (inputs quantized on the host, output
dequantized on the host): rel-err ~3e-3 << the 2e-2 gate, and it halves
both HBM traffic and DVE element count (2x_1P packed mode).

Per work tile (buffer xp = [pad/halo | x | pad/halo], 2050 cols):
  ScalarE:  t2  = Identity(xp[:, 1:2049] * w1 + b)   (center tap + bias;
            the odd-element offset breaks DVE packing, ScalarE takes it)
  VectorE:  ot  = (xp[:, 0:2048] * w0) + t2          (4B-aligned -> 2x mode)
  VectorE:  ot  = (xp[:, 2:2050] * w2) + ot          (4B-aligned -> 2x mode)

DMA: loads on the sync-engine HWDGE ring, stores + weights on the
scalar-engine ring.  Acts are issued one tile ahead of the store on the
scalar queue so a store waiting on DVE never blocks the next act
(head-of-line blocking cost the f32 baseline ~1.5us per tile).

The 4 xp buffers are persistent tiles whose pad column is zeroed once
before the loop (tiles t%2==0 pad col 0, t%2==1 pad col 2049); DMA only
ever writes the interior, so the pads stay zero across reuse.
"""

import numpy as np

B, C, L, K = 16, 512, 4096, 3
N_CORES = 8
B_SH = B // N_CORES          # 2 batches per core
NBLK = C // 128              # 4 channel blocks
NROW = B_SH * NBLK           # 8 row-tiles of [128, 4096] per core
HALF = L // 2                # row split into 2 halo-overlapped halves
NTT = NROW * 2               # 16 work tiles of [128, 2048]

_STATE = {}


def _build_program():
    from contextlib import ExitStack

    import concourse.bacc as bacc
    import concourse.mybir as mybir
    import concourse.tile as tile

    f32 = mybir.dt.float32
    bf16 = mybir.dt.bfloat16
    nc = bacc.Bacc(
        "TRN2",
        target_bir_lowering=False,
        debug=False,
        num_devices=N_CORES,
    )
    x_d = nc.dram_tensor("x", [B_SH, C, L], bf16, kind="ExternalInput").ap()
    wp_d = nc.dram_tensor("wpack", [128, 4 * NBLK], f32, kind="ExternalInput").ap()
    o_d = nc.dram_tensor("out", [B_SH, C, L], bf16, kind="ExternalOutput").ap()

    x3 = x_d.rearrange("b (k p) l -> (b k) p l", p=128)
    o3 = o_d.rearrange("b (k p) l -> (b k) p l", p=128)

    with tile.TileContext(nc) as tc, ExitStack() as ctx:
        wpool = ctx.enter_context(tc.tile_pool(name="wpool", bufs=1))
        xpool = ctx.enter_context(tc.tile_pool(name="xpool", bufs=1))
        tpool = ctx.enter_context(tc.tile_pool(name="tpool", bufs=3))
        opool = ctx.enter_context(tc.tile_pool(name="opool", bufs=3))

        wtile = wpool.tile([128, 4 * NBLK], f32)
        nc.scalar.dma_start(wtile[:, :], wp_d)

        # 4 persistent input buffers; slots 0/2 serve first halves
        # (pad col 0), slots 1/3 serve second halves (pad col HALF+1).
        xps = [xpool.tile([128, HALF + 2], bf16, name=f"xp{i}") for i in range(4)]
        nc.vector.memset(xps[0][:, 0:1], 0.0)
        nc.vector.memset(xps[2][:, 0:1], 0.0)
        nc.vector.memset(xps[1][:, HALF + 1 : HALF + 2], 0.0)
        nc.vector.memset(xps[3][:, HALF + 1 : HALF + 2], 0.0)

        def load(t):
            r, h = t // 2, t % 2
            xp = xps[t % 4]
            if h == 0:
                # [pad | x[0:HALF+1]] : out[l] needs x[l-1..l+1]
                nc.sync.dma_start(xp[:, 1 : HALF + 2], x3[r][:, 0 : HALF + 1])
            else:
                # [x[HALF-1:L] | pad]
                nc.sync.dma_start(xp[:, 0 : HALF + 1], x3[r][:, HALF - 1 : L])

        t2s = {}

        def act(t):
            blk = (t // 2) % NBLK
            xp = xps[t % 4]
            t2 = tpool.tile([128, HALF], bf16, tag="t2", name=f"t2_{t}")
            t2s[t] = t2
            nc.scalar.activation(
                t2[:, :],
                xp[:, 1 : HALF + 1],
                mybir.ActivationFunctionType.Identity,
                bias=wtile[:, blk * 4 + 3 : blk * 4 + 4],
                scale=wtile[:, blk * 4 + 1 : blk * 4 + 2],
            )

        load(0)
        load(1)
        act(0)

        for t in range(NTT):
            blk = (t // 2) % NBLK
            r, h = t // 2, t % 2
            xp = xps[t % 4]
            if t + 2 < NTT:
                load(t + 2)
            if t + 1 < NTT:
                act(t + 1)

            w0 = wtile[:, blk * 4 + 0 : blk * 4 + 1]
            w2 = wtile[:, blk * 4 + 2 : blk * 4 + 3]
            ot = opool.tile([128, HALF], bf16, tag="ot", name=f"ot_{t}")
            t2 = t2s.pop(t)
            nc.vector.scalar_tensor_tensor(
                ot[:, :],
                xp[:, 0:HALF],
                w0,
                t2[:, :],
                mybir.AluOpType.mult,
                mybir.AluOpType.add,
            )
            nc.vector.scalar_tensor_tensor(
                ot[:, :],
                xp[:, 2 : HALF + 2],
                w2,
                ot[:, :],
                mybir.AluOpType.mult,
                mybir.AluOpType.add,
            )
            nc.scalar.dma_start(o3[r][:, h * HALF : (h + 1) * HALF], ot[:, :])

    nc.compile()
    return nc


def _pack_weights(w, b):
    """[128, 4*NBLK] f32 with cols (w0, w1, w2, b) per channel block."""
    w = np.asarray(w, dtype=np.float32).reshape(C, K)
    b = np.asarray(b, dtype=np.float32)
    wp = np.zeros((128, 4 * NBLK), np.float32)
    for cb in range(NBLK):
        blk = slice(cb * 128, (cb + 1) * 128)
        wp[:, cb * 4 + 0] = w[blk, 0]
        wp[:, cb * 4 + 1] = w[blk, 1]
        wp[:, cb * 4 + 2] = w[blk, 2]
        wp[:, cb * 4 + 3] = b[blk]
    return wp


def _run(inputs, trace=False, **kw):
    import ml_dtypes

    from concourse.bass_utils import run_bass_kernel_spmd

    if "nc" not in _STATE:
        _STATE["nc"] = _build_program()
    nc = _STATE["nc"]

    x = np.ascontiguousarray(
        np.asarray(inputs["x"], dtype=np.float32).astype(ml_dtypes.bfloat16)
    )
    wp = _pack_weights(inputs["w"], inputs["b"])
    in_maps = [
        {"x": x[c * B_SH : (c + 1) * B_SH], "wpack": wp} for c in range(N_CORES)
    ]
    res = run_bass_kernel_spmd(
        nc, in_maps, core_ids=list(range(N_CORES)), trace=trace, **kw
    )
    out = np.concatenate([res.results[c]["out"] for c in range(N_CORES)], axis=0)
    return out.astype(np.float32), res


def kernel(**inputs):
    return _run(inputs)[0]


# revision 10
# speedup vs baseline: 1.1112x; 1.1112x over previous
"""Depthwise Conv1d (C=512, K=3, stride=1, pad=1) on 8 Trainium2 NeuronCores.

Problem: x [16, 512, 4096] f32, w [512, 1, 3] f32, b [512] f32
         out[n,c,l] = sum_k w[c,0,k] * x_pad[n,c,l+k] + b[c]

Sharding: data-parallel over batch — 2 batches per core; each core handles
all 512 channels as 4 blocks of 128 partitions -> 8 rows of [128, 4096].

Computes in bf16 (host quantizes inputs / dequantizes the output):
rel-err ~3e-3 << the 2e-2 gate, and it halves HBM traffic and doubles
DVE throughput.  The host also bakes the two zero pad columns into the
uploaded tensor, so a row load is one contiguous [128, 4098] transfer
and no on-device memsets are needed.

Engine assignment per row (HW-measured op costs at FD=4096):
  ScalarE  tC = Identity(x1*w1 + b)              (~3.8us, 1 elem/cyc)
  VectorE  tA = ts(x0*w0)      4x mode (~1.2us)
           tB = ts(x2*w2)      4x mode (~1.2us)  [rows 2,5: on ScalarE]
           s  = tt(tA + tC)    2x mode (~2.3us)  [rows 1-4: on GpSimd ~7us]
           ot = tt(s + tB)     2x mode (~2.3us)
  (scalar_tensor_tensor has NO fast DVE uop -> 1x; avoided entirely.)
Adds are computed in-place into tA to save SBUF.  Rows 0 and 7 run in
half-row chunks to shorten the pipeline head and tail.

DMA: loads then stores on the sync-engine HWDGE ring (program order:
all loads first, stores ordered by expected completion); weights on the
scalar-engine ring so the scalar queue is never blocked by a store.
"""

import numpy as np

B, C, L, K = 16, 512, 4096, 3
N_CORES = 8
B_SH = B // N_CORES          # 2 batches per core
NBLK = C // 128              # 4 channel blocks
NROW = B_SH * NBLK           # 8 rows of [128, 4096] per core
HALF = L // 2
LP = L + 2                   # padded row length

GP_ROWS = (1, 2, 3, 4)       # first add on GpSimd
SCALB_ROWS = (2, 5)          # w2 tap on ScalarE instead of DVE

_STATE = {}


def _build_program():
    from contextlib import ExitStack

    import concourse.bacc as bacc
    import concourse.mybir as mybir
    import concourse.tile as tile

    f32 = mybir.dt.float32
    bf16 = mybir.dt.bfloat16
    MULT = mybir.AluOpType.mult
    ADD = mybir.AluOpType.add
    IDENT = mybir.ActivationFunctionType.Identity

    nc = bacc.Bacc(
        "TRN2",
        target_bir_lowering=False,
        debug=False,
        num_devices=N_CORES,
    )
    x_d = nc.dram_tensor("x", [B_SH, C, LP], bf16, kind="ExternalInput").ap()
    wp_d = nc.dram_tensor("wpack", [128, 4 * NBLK], f32, kind="ExternalInput").ap()
    o_d = nc.dram_tensor("out", [B_SH, C, L], bf16, kind="ExternalOutput").ap()

    x3 = x_d.rearrange("b (k p) l -> (b k) p l", p=128)
    o3 = o_d.rearrange("b (k p) l -> (b k) p l", p=128)

    with tile.TileContext(nc) as tc, ExitStack() as ctx:
        wpool = ctx.enter_context(tc.tile_pool(name="wpool", bufs=1))
        xpool = ctx.enter_context(tc.tile_pool(name="xpool", bufs=3))
        apool = ctx.enter_context(tc.tile_pool(name="apool", bufs=6))
        bpool = ctx.enter_context(tc.tile_pool(name="bpool", bufs=6))
        cpool = ctx.enter_context(tc.tile_pool(name="cpool", bufs=4))
        hpool = ctx.enter_context(tc.tile_pool(name="hpool", bufs=2))

        wtile = wpool.tile([128, 4 * NBLK], f32)
        nc.scalar.dma_start(wtile[:, :], wp_d)

        def wsl(r, j):  # w0/w1/w2/bias column for row r's channel block
            blk = r % NBLK
            return wtile[:, blk * 4 + j : blk * 4 + j + 1]

        xps = {}

        def load(r, half=False):
            xps[r] = xpool.tile([128, LP], bf16, tag="xp", name=f"xp{r}")
            if half:
                # split at element 2050 so each half-op's operands are
                # covered by its own load
                nc.sync.dma_start(xps[r][:, 0:2050], x3[r][:, 0:2050])
                nc.sync.dma_start(xps[r][:, 2050:LP], x3[r][:, 2050:LP])
            else:
                nc.sync.dma_start(xps[r][:, :], x3[r])

        tC = {}
        tB = {}

        def actC(r, lo, n, tag, name):
            t = cpool.tile(
                [128, n], bf16, tag=tag, name=name,
                bufs=2 if tag == "tCh" else None,
            )
            nc.scalar.activation(
                t[:, :], xps[r][:, lo + 1 : lo + 1 + n],
                IDENT, bias=wsl(r, 3), scale=wsl(r, 1),
            )
            return t

        def actB(r):
            t = bpool.tile([128, L], bf16, tag="tB", name=f"tB{r}")
            nc.scalar.activation(
                t[:, :], xps[r][:, 2:LP], IDENT, scale=wsl(r, 2),
            )
            return t

        # ---------- DVE / GpSimd compute ----------
        ots = {}

        def ts_full(r, j, pool, tag, name):
            t = pool.tile([128, L], bf16, tag=tag, name=name)
            nc.vector.tensor_scalar(
                t[:, :], xps[r][:, j : j + L], wsl(r, j), None, MULT
            )
            return t

        def row_half(r, h):
            """full chain for one half of row r on DVE; returns ot tile"""
            lo = h * HALF
            xp = xps[r]
            ta = hpool.tile([128, HALF], bf16, tag="tAh", name=f"tA{r}{h}")
            nc.vector.tensor_scalar(
                ta[:, :], xp[:, lo : lo + HALF], wsl(r, 0), None, MULT
            )
            tb = hpool.tile([128, HALF], bf16, tag="tBh", name=f"tB{r}{h}")
            nc.vector.tensor_scalar(
                tb[:, :], xp[:, lo + 2 : lo + 2 + HALF], wsl(r, 2), None, MULT
            )
            nc.vector.tensor_tensor(ta[:, :], ta[:, :], tC[(r, h)][:, :], ADD)
            nc.vector.tensor_tensor(ta[:, :], ta[:, :], tb[:, :], ADD)
            return ta

        def dve_add(dst, other):
            nc.vector.tensor_tensor(dst[:, :], dst[:, :], other[:, :], ADD)

        def gp_s(r):
            # GpSimd: s_r = tA_r + tC_r in-place into tA
            nc.gpsimd.tensor_tensor(tA[r][:, :], tA[r][:, :], tC[r][:, :], ADD)

        def store(r, h=None):
            if h is None:
                nc.sync.dma_start(o3[r], ots[r][:, :])
            else:
                lo = h * HALF
                nc.sync.dma_start(o3[r][:, lo : lo + HALF], ots[(r, h)][:, :])

        tA = {}
        # ---- emission in pipeline-flow order (per-queue program order ----
        # ---- is the schedule; DMA-completion sems are batched per ring) ----
        load(0, half=True)
        load(1)
        load(2)
        tC[(0, 0)] = actC(0, 0, HALF, "tCh", "tC0a")
        tC[(0, 1)] = actC(0, HALF, HALF, "tCh", "tC0b")
        tC[1] = actC(1, 0, L, "tC", "tC1")
        tA[1] = ts_full(1, 0, apool, "tA", "tA1")
        tB[1] = ts_full(1, 2, bpool, "tB", "tB1")
        gp_s(1)
        ots[(0, 0)] = row_half(0, 0)
        ots[(0, 1)] = row_half(0, 1)
        load(3)
        tC[2] = actC(2, 0, L, "tC", "tC2")
        tB[2] = actB(2)
        tA[2] = ts_full(2, 0, apool, "tA", "tA2")
        gp_s(2)
        load(4)
        tC[3] = actC(3, 0, L, "tC", "tC3")
        tA[3] = ts_full(3, 0, apool, "tA", "tA3")
        tB[3] = ts_full(3, 2, bpool, "tB", "tB3")
        gp_s(3)
        load(5)
        store(0, 0)
        store(0, 1)
        tC[4] = actC(4, 0, L, "tC", "tC4")
        tA[4] = ts_full(4, 0, apool, "tA", "tA4")
        tB[4] = ts_full(4, 2, bpool, "tB", "tB4")
        gp_s(4)
        load(6)
        tC[5] = actC(5, 0, L, "tC", "tC5")
        tB[5] = actB(5)
        dve_add(tA[1], tB[1])             # ot1
        ots[1] = tA[1]
        tA[5] = ts_full(5, 0, apool, "tA", "tA5")
        load(7, half=True)
        tC[6] = actC(6, 0, L, "tC", "tC6")
        dve_add(tA[2], tB[2])             # ot2
        ots[2] = tA[2]
        tA[6] = ts_full(6, 0, apool, "tA", "tA6")
        tB[6] = ts_full(6, 2, bpool, "tB", "tB6")
        dve_add(tA[3], tB[3])             # ot3
        ots[3] = tA[3]
        tC[(7, 0)] = actC(7, 0, HALF, "tCh", "tC7a")
        tC[(7, 1)] = actC(7, HALF, HALF, "tCh", "tC7b")
        dve_add(tA[5], tC[5])             # s5 on DVE
        dve_add(tA[5], tB[5])             # ot5
        ots[5] = tA[5]
        dve_add(tA[6], tC[6])             # s6
        dve_add(tA[4], tB[4])             # ot4
        ots[4] = tA[4]
        dve_add(tA[6], tB[6])             # ot6
        ots[6] = tA[6]
        ots[(7, 0)] = row_half(7, 0)
        ots[(7, 1)] = row_half(7, 1)
        for args in [(1,), (2,), (3,), (5,), (4,), (6,), (7, 0), (7, 1)]:
            store(*args)

    nc.compile()
    return nc


def _pack_weights(w, b):
    """[128, 4*NBLK] f32 with cols (w0, w1, w2, b) per channel block."""
    w = np.asarray(w, dtype=np.float32).reshape(C, K)
    b = np.asarray(b, dtype=np.float32)
    wp = np.zeros((128, 4 * NBLK), np.float32)
    for cb in range(NBLK):
        blk = slice(cb * 128, (cb + 1) * 128)
        wp[:, cb * 4 + 0] = w[blk, 0]
        wp[:, cb * 4 + 1] = w[blk, 1]
        wp[:, cb * 4 + 2] = w[blk, 2]
        wp[:, cb * 4 + 3] = b[blk]
    return wp


def _run(inputs, trace=False, **kw):
    import ml_dtypes

    from concourse.bass_utils import run_bass_kernel_spmd

    if "nc" not in _STATE:
        _STATE["nc"] = _build_program()
    nc = _STATE["nc"]

    x = np.asarray(inputs["x"], dtype=np.float32)
    xq = np.zeros((B, C, LP), dtype=ml_dtypes.bfloat16)
    xq[:, :, 1 : L + 1] = x.astype(ml_dtypes.bfloat16)
    wp = _pack_weights(inputs["w"], inputs["b"])
    in_maps = [
        {"x": xq[c * B_SH : (c + 1) * B_SH], "wpack": wp} for c in range(N_CORES)
    ]
    res = run_bass_kernel_spmd(
        nc, in_maps, core_ids=list(range(N_CORES)), trace=trace, **kw
    )
    out = np.concatenate([res.results[c]["out"] for c in range(N_CORES)], axis=0)
    return out.astype(np.float32), res


def kernel(**inputs):
    return _run(inputs)[0]


# revision 11
# speedup vs baseline: 1.3271x; 1.1943x over previous
"""Depthwise Conv1d (C=512, K=3, stride=1, pad=1) on 8 Trainium2 NeuronCores.

Problem: x [16, 512, 4096] f32, w [512, 1, 3] f32, b [512] f32
         out[n,c,l] = sum_k w[c,0,k] * x_pad[n,c,l+k] + b[c]

Sharding: data-parallel over batch — 2 batches per core; each core handles
all 512 channels as 4 blocks of 128 partitions -> 8 rows of [128, 4096].

Computes in bf16 (host quantizes inputs / dequantizes the output):
rel-err ~3e-3 << the 2e-2 gate.  The host also bakes the two zero pad
columns into the uploaded tensor, so a row load is one contiguous
[128, 4098] transfer and no on-device memsets are needed.

Work is split across three engines (HW-measured op costs):

DVE+ScalarE rows (0, 1, 5, 6, 7):
  ScalarE  tC = Identity(x1*w1 + b)          ~3.8us  (1 elem/cyc)
  VectorE  tA = ts(x0*w0)   4x mode  ~1.27us
           tB = ts(x2*w2)   4x mode  ~1.27us  (row 5: on ScalarE)
           s  = tt(tA+tC)   2x mode  ~2.28us
           ot = tt(s+tB)    2x mode  ~2.28us  (adds in-place into tA)
  (scalar_tensor_tensor has NO fast DVE uop -> 1x; avoided entirely.
   GpSimd tensor_tensor works but contends for the DVE SBUF port and
   slows concurrent DVE ops 4-7x -> not used.)

TensorE rows (2, 3, 4), back-to-back so the PE stays at full p-state:
  per half-row [128,2048]: 4 col-chunks x 3 taps of diag(w_k) matmuls
  accumulating in PSUM f32 (moving free dim capped at 512), then
  ScalarE evacuates: ot = Identity(psum + b) -> SBUF bf16 (~1.85us).

DMA: loads + stores interleaved on the sync-engine HWDGE ring in
pipeline-flow order; weights on the scalar-engine ring.  Rows 0 and 7
run in half-row chunks to shorten the pipeline head and tail.
"""

import numpy as np

B, C, L, K = 16, 512, 4096, 3
N_CORES = 8
B_SH = B // N_CORES          # 2 batches per core
NBLK = C // 128              # 4 channel blocks
NROW = B_SH * NBLK           # 8 rows of [128, 4096] per core
HALF = L // 2
LP = L + 2                   # padded row length
MM = 512                     # max moving free dim per matmul

_STATE = {}


def _build_program():
    from contextlib import ExitStack

    import concourse.bacc as bacc
    import concourse.mybir as mybir
    import concourse.tile as tile

    f32 = mybir.dt.float32
    bf16 = mybir.dt.bfloat16
    MULT = mybir.AluOpType.mult
    ADD = mybir.AluOpType.add
    IDENT = mybir.ActivationFunctionType.Identity

    nc = bacc.Bacc(
        "TRN2",
        target_bir_lowering=False,
        debug=False,
        num_devices=N_CORES,
    )
    x_d = nc.dram_tensor("x", [B_SH, C, LP], bf16, kind="ExternalInput").ap()
    wp_d = nc.dram_tensor("wpack", [128, 4 * NBLK], f32, kind="ExternalInput").ap()
    wd_d = nc.dram_tensor(
        "wdiag", [128, 3 * NBLK * 128], bf16, kind="ExternalInput"
    ).ap()
    o_d = nc.dram_tensor("out", [B_SH, C, L], bf16, kind="ExternalOutput").ap()

    x3 = x_d.rearrange("b (k p) l -> (b k) p l", p=128)
    o3 = o_d.rearrange("b (k p) l -> (b k) p l", p=128)

    with tile.TileContext(nc) as tc, ExitStack() as ctx:
        wpool = ctx.enter_context(tc.tile_pool(name="wpool", bufs=1))
        xpool = ctx.enter_context(tc.tile_pool(name="xpool", bufs=3))
        apool = ctx.enter_context(tc.tile_pool(name="apool", bufs=3))
        bpool = ctx.enter_context(tc.tile_pool(name="bpool", bufs=3))
        cpool = ctx.enter_context(tc.tile_pool(name="cpool", bufs=3))
        hpool = ctx.enter_context(tc.tile_pool(name="hpool", bufs=2))
        epool = ctx.enter_context(tc.tile_pool(name="epool", bufs=4))
        pspool = ctx.enter_context(tc.tile_pool(name="pspool", bufs=2, space="PSUM"))

        wtile = wpool.tile([128, 4 * NBLK], f32)
        nc.scalar.dma_start(wtile[:, :], wp_d)
        wdiag = wpool.tile([128, 3 * NBLK * 128], bf16)
        nc.scalar.dma_start(wdiag[:, :], wd_d)

        def wsl(r, j):  # w0/w1/w2/bias column for row r's channel block
            blk = r % NBLK
            return wtile[:, blk * 4 + j : blk * 4 + j + 1]

        def wdg(r, k):  # [128,128] diag(w_k) for row r's channel block
            blk = r % NBLK
            o = (blk * 3 + k) * 128
            return wdiag[:, o : o + 128]

        xps = {}

        def load(r, half=False):
            xps[r] = xpool.tile([128, LP], bf16, tag="xp", name=f"xp{r}")
            if half:
                # split at element 2050 so each half-op's operands are
                # covered by its own load
                nc.sync.dma_start(xps[r][:, 0:2050], x3[r][:, 0:2050])
                nc.sync.dma_start(xps[r][:, 2050:LP], x3[r][:, 2050:LP])
            else:
                nc.sync.dma_start(xps[r][:, :], x3[r])

        tA = {}
        tB = {}
        tC = {}
        ots = {}

        def actC(r, lo, n, tag, name):
            t = cpool.tile(
                [128, n], bf16, tag=tag, name=name,
                bufs=2 if tag == "tCh" else None,
            )
            nc.scalar.activation(
                t[:, :], xps[r][:, lo + 1 : lo + 1 + n],
                IDENT, bias=wsl(r, 3), scale=wsl(r, 1),
            )
            return t

        def actB(r):
            t = bpool.tile([128, L], bf16, tag="tB", name=f"tB{r}")
            nc.scalar.activation(
                t[:, :], xps[r][:, 2:LP], IDENT, scale=wsl(r, 2),
            )
            return t

        def ts_full(r, j, pool, tag, name):
            t = pool.tile([128, L], bf16, tag=tag, name=name)
            nc.vector.tensor_scalar(
                t[:, :], xps[r][:, j : j + L], wsl(r, j), None, MULT
            )
            return t

        def row_half(r, h):
            """full chain for one half of row r on DVE; returns ot tile"""
            lo = h * HALF
            xp = xps[r]
            ta = hpool.tile([128, HALF], bf16, tag="tAh", name=f"tA{r}{h}")
            nc.vector.tensor_scalar(
                ta[:, :], xp[:, lo : lo + HALF], wsl(r, 0), None, MULT
            )
            tb = hpool.tile([128, HALF], bf16, tag="tBh", name=f"tB{r}{h}")
            nc.vector.tensor_scalar(
                tb[:, :], xp[:, lo + 2 : lo + 2 + HALF], wsl(r, 2), None, MULT
            )
            nc.vector.tensor_tensor(ta[:, :], ta[:, :], tC[(r, h)][:, :], ADD)
            nc.vector.tensor_tensor(ta[:, :], ta[:, :], tb[:, :], ADD)
            return ta

        def dve_add(dst, other):
            nc.vector.tensor_tensor(dst[:, :], dst[:, :], other[:, :], ADD)

        def pe_half(r, h):
            """12 accumulating diag matmuls for one half of row r -> psum"""
            lo = h * HALF
            ps = pspool.tile([128, HALF], f32, tag="ps", name=f"ps{r}{h}")
            for c in range(HALF // MM):
                for k in range(3):
                    nc.tensor.matmul(
                        ps[:, c * MM : (c + 1) * MM],
                        wdg(r, k),
                        xps[r][:, lo + k + c * MM : lo + k + c * MM + MM],
                        start=(k == 0),
                        stop=(k == 2),
                    )
            return ps

        def evac(r, h, ps):
            t = epool.tile([128, HALF], bf16, tag="ev", name=f"ev{r}{h}")
            nc.scalar.activation(
                t[:, :], ps[:, :], IDENT, bias=wsl(r, 3), scale=1.0,
            )
            ots[(r, h)] = t
            return t

        def store(r, h=None):
            if h is None:
                nc.sync.dma_start(o3[r], ots[r][:, :])
            else:
                lo = h * HALF
                nc.sync.dma_start(o3[r][:, lo : lo + HALF], ots[(r, h)][:, :])

        # ---- emission in pipeline-flow order (per-queue program order ----
        # ---- is the schedule; DMA-completion sems are batched per ring) ----
        load(0, half=True)
        load(1)
        load(2)
        tC[(0, 0)] = actC(0, 0, HALF, "tCh", "tC0a")
        tC[(0, 1)] = actC(0, HALF, HALF, "tCh", "tC0b")
        tC[1] = actC(1, 0, L, "tC", "tC1")
        tA[1] = ts_full(1, 0, apool, "tA", "tA1")
        tB[1] = ts_full(1, 2, bpool, "tB", "tB1")
        ots[(0, 0)] = row_half(0, 0)
        ots[(0, 1)] = row_half(0, 1)
        dve_add(tA[1], tC[1])             # s1
        dve_add(tA[1], tB[1])             # ot1
        ots[1] = tA[1]
        load(3)
        ps = pe_half(2, 0)
        evac(2, 0, ps)
        ps = pe_half(2, 1)
        evac(2, 1, ps)
        load(4)
        store(0, 0)
        store(0, 1)
        store(1)
        ps = pe_half(3, 0)
        evac(3, 0, ps)
        ps = pe_half(3, 1)
        evac(3, 1, ps)
        load(5)
        tC[5] = actC(5, 0, L, "tC", "tC5")
        tB[5] = actB(5)
        tA[5] = ts_full(5, 0, apool, "tA", "tA5")
        store(2, 0)
        store(2, 1)
        ps = pe_half(4, 0)
        evac(4, 0, ps)
        ps = pe_half(4, 1)
        evac(4, 1, ps)
        load(6)
        tC[6] = actC(6, 0, L, "tC", "tC6")
        dve_add(tA[5], tC[5])             # s5
        dve_add(tA[5], tB[5])             # ot5
        ots[5] = tA[5]
        tA[6] = ts_full(6, 0, apool, "tA", "tA6")
        tB[6] = ts_full(6, 2, bpool, "tB", "tB6")
        load(7, half=True)
        store(3, 0)
        store(3, 1)
        tC[(7, 0)] = actC(7, 0, HALF, "tCh", "tC7a")
        tC[(7, 1)] = actC(7, HALF, HALF, "tCh", "tC7b")
        dve_add(tA[6], tC[6])             # s6
        dve_add(tA[6], tB[6])             # ot6
        ots[6] = tA[6]
        store(4, 0)
        store(4, 1)
        store(5)
        ots[(7, 0)] = row_half(7, 0)
        ots[(7, 1)] = row_half(7, 1)
        store(6)
        store(7, 0)
        store(7, 1)

    nc.compile()
    return nc


def _pack_weights(w, b):
    """[128, 4*NBLK] f32 with cols (w0, w1, w2, b) per channel block."""
    w = np.asarray(w, dtype=np.float32).reshape(C, K)
    b = np.asarray(b, dtype=np.float32)
    wp = np.zeros((128, 4 * NBLK), np.float32)
    for cb in range(NBLK):
        blk = slice(cb * 128, (cb + 1) * 128)
        wp[:, cb * 4 + 0] = w[blk, 0]
        wp[:, cb * 4 + 1] = w[blk, 1]
        wp[:, cb * 4 + 2] = w[blk, 2]
        wp[:, cb * 4 + 3] = b[blk]
    return wp


def _pack_diag(w):
    """[128, 3*NBLK*128] bf16: diag(w_k) per (block, tap)."""
    import ml_dtypes

    w = np.asarray(w, dtype=np.float32).reshape(C, K)
    wd = np.zeros((128, 3 * NBLK * 128), np.float32)
    for cb in range(NBLK):
        for k in range(3):
            o = (cb * 3 + k) * 128
            wd[np.arange(128), o + np.arange(128)] = w[cb * 128 : (cb + 1) * 128, k]
    return wd.astype(ml_dtypes.bfloat16)


def _run(inputs, trace=False, **kw):
    import ml_dtypes

    from concourse.bass_utils import run_bass_kernel_spmd

    if "nc" not in _STATE:
        _STATE["nc"] = _build_program()
    nc = _STATE["nc"]

    x = np.asarray(inputs["x"], dtype=np.float32)
    xq = np.zeros((B, C, LP), dtype=ml_dtypes.bfloat16)
    xq[:, :, 1 : L + 1] = x.astype(ml_dtypes.bfloat16)
    wp = _pack_weights(inputs["w"], inputs["b"])
    wd = _pack_diag(inputs["w"])
    in_maps = [
        {"x": xq[c * B_SH : (c + 1) * B_SH], "wpack": wp, "wdiag": wd}
        for c in range(N_CORES)
    ]
    res = run_bass_kernel_spmd(
        nc, in_maps, core_ids=list(range(N_CORES)), trace=trace, **kw
    )
    out = np.concatenate([res.results[c]["out"] for c in range(N_CORES)], axis=0)
    return out.astype(np.float32), res


def kernel(**inputs):
    return _run(inputs)[0]


# revision 13
# speedup vs baseline: 1.4273x; 1.0755x over previous
"""Depthwise Conv1d (C=512, K=3, stride=1, pad=1) on 8 Trainium2 NeuronCores.

Problem: x [16, 512, 4096] f32, w [512, 1, 3] f32, b [512] f32
         out[n,c,l] = sum_k w[c,0,k] * x_pad[n,c,l+k] + b[c]

Sharding: data-parallel over batch — 2 batches per core; each core handles
all 512 channels as 4 blocks of 128 partitions -> 8 rows of [128, 4096].

Computes in bf16 (host quantizes inputs / dequantizes the output):
rel-err ~3e-3 << the 2e-2 gate.  The host also bakes the two zero pad
columns into the uploaded tensor, so a row load is one contiguous
[128, 4098] transfer and no on-device memsets are needed.

Work is split across three engines (HW-measured op costs, including
the ~20% slowdown all engines see while DMA traffic is in flight):

PE rows (0, 2, 3, 4) and DVE+ScalarE rows (1, 5, 6, 7):
  ScalarE  tC = Identity(x1*w1 + b)          ~3.8us  (1 elem/cyc)
  VectorE  tA = ts(x0*w0)   4x mode  ~1.27us
           tB = ts(x2*w2)   4x mode  ~1.27us  (row 5: on ScalarE)
           s  = tt(tA+tC)   2x mode  ~2.28us
           ot = tt(s+tB)    2x mode  ~2.28us  (adds in-place into tA)
  (scalar_tensor_tensor has NO fast DVE uop -> 1x; avoided entirely.
   GpSimd tensor_tensor works but contends for the DVE SBUF port and
   slows concurrent DVE ops 4-7x -> not used.)

TensorE rows (2, 3, 4), back-to-back so the PE stays at full p-state:
  per half-row [128,2048]: 4 col-chunks x 3 taps of diag(w_k) matmuls
  accumulating in PSUM f32 (moving free dim capped at 512), then
  ScalarE evacuates: ot = Identity(psum + b) -> SBUF bf16 (~1.85us).

DMA: loads + stores interleaved on the sync-engine HWDGE ring in
pipeline-flow order; weights on the scalar-engine ring.  Rows 0 and 7
run in half-row chunks to shorten the pipeline head and tail.
"""

import numpy as np

B, C, L, K = 16, 512, 4096, 3
N_CORES = 8
B_SH = B // N_CORES          # 2 batches per core
NBLK = C // 128              # 4 channel blocks
NROW = B_SH * NBLK           # 8 rows of [128, 4096] per core
HALF = L // 2
LP = L + 2                   # padded row length
MM = 512                     # max moving free dim per matmul

_STATE = {}


def _build_program():
    from contextlib import ExitStack

    import concourse.bacc as bacc
    import concourse.mybir as mybir
    import concourse.tile as tile

    f32 = mybir.dt.float32
    bf16 = mybir.dt.bfloat16
    MULT = mybir.AluOpType.mult
    ADD = mybir.AluOpType.add
    IDENT = mybir.ActivationFunctionType.Identity

    nc = bacc.Bacc(
        "TRN2",
        target_bir_lowering=False,
        debug=False,
        num_devices=N_CORES,
    )
    x_d = nc.dram_tensor("x", [B_SH, C, LP], bf16, kind="ExternalInput").ap()
    wp_d = nc.dram_tensor("wpack", [128, 4 * NBLK], f32, kind="ExternalInput").ap()
    wd_d = nc.dram_tensor(
        "wdiag", [128, 3 * NBLK * 128], bf16, kind="ExternalInput"
    ).ap()
    o_d = nc.dram_tensor("out", [B_SH, C, L], bf16, kind="ExternalOutput").ap()

    x3 = x_d.rearrange("b (k p) l -> (b k) p l", p=128)
    o3 = o_d.rearrange("b (k p) l -> (b k) p l", p=128)

    with tile.TileContext(nc) as tc, ExitStack() as ctx:
        wpool = ctx.enter_context(tc.tile_pool(name="wpool", bufs=1))
        xpool = ctx.enter_context(tc.tile_pool(name="xpool", bufs=3))
        apool = ctx.enter_context(tc.tile_pool(name="apool", bufs=3))
        bpool = ctx.enter_context(tc.tile_pool(name="bpool", bufs=3))
        cpool = ctx.enter_context(tc.tile_pool(name="cpool", bufs=3))
        hpool = ctx.enter_context(tc.tile_pool(name="hpool", bufs=2))
        epool = ctx.enter_context(tc.tile_pool(name="epool", bufs=4))
        pspool = ctx.enter_context(tc.tile_pool(name="pspool", bufs=2, space="PSUM"))

        wtile = wpool.tile([128, 4 * NBLK], f32)
        nc.scalar.dma_start(wtile[:, :], wp_d)
        wdiag = wpool.tile([128, 3 * NBLK * 128], bf16)
        nc.scalar.dma_start(wdiag[:, :], wd_d)

        def wsl(r, j):  # w0/w1/w2/bias column for row r's channel block
            blk = r % NBLK
            return wtile[:, blk * 4 + j : blk * 4 + j + 1]

        def wdg(r, k):  # [128,128] diag(w_k) for row r's channel block
            blk = r % NBLK
            o = (blk * 3 + k) * 128
            return wdiag[:, o : o + 128]

        xps = {}

        def load(r, half=False):
            xps[r] = xpool.tile([128, LP], bf16, tag="xp", name=f"xp{r}")
            if half:
                # split at element 2050 so each half-op's operands are
                # covered by its own load
                nc.sync.dma_start(xps[r][:, 0:2050], x3[r][:, 0:2050])
                nc.sync.dma_start(xps[r][:, 2050:LP], x3[r][:, 2050:LP])
            else:
                nc.sync.dma_start(xps[r][:, :], x3[r])

        tA = {}
        tB = {}
        tC = {}
        ots = {}

        def actC(r, lo, n, tag, name):
            t = cpool.tile(
                [128, n], bf16, tag=tag, name=name,
                bufs=2 if tag == "tCh" else None,
            )
            nc.scalar.activation(
                t[:, :], xps[r][:, lo + 1 : lo + 1 + n],
                IDENT, bias=wsl(r, 3), scale=wsl(r, 1),
            )
            return t

        def actB(r):
            t = bpool.tile([128, L], bf16, tag="tB", name=f"tB{r}")
            nc.scalar.activation(
                t[:, :], xps[r][:, 2:LP], IDENT, scale=wsl(r, 2),
            )
            return t

        def ts_full(r, j, pool, tag, name):
            t = pool.tile([128, L], bf16, tag=tag, name=name)
            nc.vector.tensor_scalar(
                t[:, :], xps[r][:, j : j + L], wsl(r, j), None, MULT
            )
            return t

        def row_half(r, h):
            """full chain for one half of row r on DVE; returns ot tile"""
            lo = h * HALF
            xp = xps[r]
            ta = hpool.tile([128, HALF], bf16, tag="tAh", name=f"tA{r}{h}")
            nc.vector.tensor_scalar(
                ta[:, :], xp[:, lo : lo + HALF], wsl(r, 0), None, MULT
            )
            tb = hpool.tile([128, HALF], bf16, tag="tBh", name=f"tB{r}{h}")
            nc.vector.tensor_scalar(
                tb[:, :], xp[:, lo + 2 : lo + 2 + HALF], wsl(r, 2), None, MULT
            )
            nc.vector.tensor_tensor(ta[:, :], ta[:, :], tC[(r, h)][:, :], ADD)
            nc.vector.tensor_tensor(ta[:, :], ta[:, :], tb[:, :], ADD)
            return ta

        def dve_add(dst, other):
            nc.vector.tensor_tensor(dst[:, :], dst[:, :], other[:, :], ADD)

        def pe_half(r, h):
            """12 accumulating diag matmuls for one half of row r -> psum"""
            lo = h * HALF
            ps = pspool.tile([128, HALF], f32, tag="ps", name=f"ps{r}{h}")
            for c in range(HALF // MM):
                for k in range(3):
                    nc.tensor.matmul(
                        ps[:, c * MM : (c + 1) * MM],
                        wdg(r, k),
                        xps[r][:, lo + k + c * MM : lo + k + c * MM + MM],
                        start=(k == 0),
                        stop=(k == 2),
                    )
            return ps

        def evac(r, h, ps):
            t = epool.tile([128, HALF], bf16, tag="ev", name=f"ev{r}{h}")
            nc.scalar.activation(
                t[:, :], ps[:, :], IDENT, bias=wsl(r, 3), scale=1.0,
            )
            ots[(r, h)] = t
            return t

        def store(r, h=None):
            if h is None:
                nc.sync.dma_start(o3[r], ots[r][:, :])
            else:
                lo = h * HALF
                nc.sync.dma_start(o3[r][:, lo : lo + HALF], ots[(r, h)][:, :])

        def row_half_dve(r, h):
            """all-DVE half chain: center tap via 2-scalar ts (w1*x + b)"""
            lo = h * HALF
            xp = xps[r]
            ta = hpool.tile([128, HALF], bf16, tag="tAh", name=f"tA{r}{h}")
            nc.vector.tensor_scalar(
                ta[:, :], xp[:, lo : lo + HALF], wsl(r, 0), None, MULT
            )
            tb = hpool.tile([128, HALF], bf16, tag="tBh", name=f"tB{r}{h}")
            nc.vector.tensor_scalar(
                tb[:, :], xp[:, lo + 2 : lo + 2 + HALF], wsl(r, 2), None, MULT
            )
            tc = hpool.tile([128, HALF], bf16, tag="tCh2", name=f"tCd{r}{h}")
            nc.vector.tensor_scalar(
                tc[:, :], xp[:, lo + 1 : lo + 1 + HALF], wsl(r, 1), wsl(r, 3),
                MULT, ADD,
            )
            nc.vector.tensor_tensor(ta[:, :], ta[:, :], tc[:, :], ADD)
            nc.vector.tensor_tensor(ta[:, :], ta[:, :], tb[:, :], ADD)
            return ta

        # ---- emission in pipeline-flow order (per-queue program order ----
        # ---- is the schedule; DMA-completion sems are batched per ring) ----
        # loads: 0,1,2,7,5,3,4,6 so the DVE tail rows' data lands early
        load(0, half=True)
        load(1)
        # row 1: all-DVE (center tap via 2-scalar ts -> no scalar dep)
        tA[1] = ts_full(1, 0, apool, "tA", "tA1")
        tB[1] = ts_full(1, 2, bpool, "tB", "tB1")
        tC[1] = cpool.tile([128, L], bf16, tag="tC", name="tC1")
        nc.vector.tensor_scalar(
            tC[1][:, :], xps[1][:, 1 : 1 + L], wsl(1, 1), wsl(1, 3), MULT, ADD
        )
        dve_add(tA[1], tC[1])             # s1
        dve_add(tA[1], tB[1])             # ot1
        ots[1] = tA[1]
        load(2)
        ps = pe_half(0, 0)
        evac(0, 0, ps)
        ps = pe_half(0, 1)
        evac(0, 1, ps)
        load(7, half=True)
        ps = pe_half(2, 0)
        evac(2, 0, ps)
        load(5)
        tC[5] = actC(5, 0, L, "tC", "tC5")
        ps = pe_half(2, 1)
        evac(2, 1, ps)
        store(0, 0)
        store(0, 1)
        store(1)
        # row 7: all-DVE halves (tail rows computed early thanks to ld7)
        ots[(7, 0)] = row_half_dve(7, 0)
        ots[(7, 1)] = row_half_dve(7, 1)
        load(3)
        tA[5] = ts_full(5, 0, apool, "tA", "tA5")
        tB[5] = ts_full(5, 2, bpool, "tB", "tB5")
        dve_add(tA[5], tC[5])             # s5
        dve_add(tA[5], tB[5])             # ot5
        ots[5] = tA[5]
        ps = pe_half(3, 0)
        evac(3, 0, ps)
        ps = pe_half(3, 1)
        evac(3, 1, ps)
        load(4)
        store(7, 0)
        store(7, 1)
        store(2, 0)
        store(2, 1)
        load(6)
        tC[6] = actC(6, 0, L, "tC", "tC6")
        tB[6] = actB(6)
        tA[6] = ts_full(6, 0, apool, "tA", "tA6")
        ps = pe_half(4, 0)
        evac(4, 0, ps)
        ps = pe_half(4, 1)
        evac(4, 1, ps)
        dve_add(tA[6], tC[6])             # s6
        dve_add(tA[6], tB[6])             # ot6
        ots[6] = tA[6]
        store(5)
        store(3, 0)
        store(3, 1)
        store(4, 0)
        store(4, 1)
        store(6)

    nc.compile()
    return nc


def _pack_weights(w, b):
    """[128, 4*NBLK] f32 with cols (w0, w1, w2, b) per channel block."""
    w = np.asarray(w, dtype=np.float32).reshape(C, K)
    b = np.asarray(b, dtype=np.float32)
    wp = np.zeros((128, 4 * NBLK), np.float32)
    for cb in range(NBLK):
        blk = slice(cb * 128, (cb + 1) * 128)
        wp[:, cb * 4 + 0] = w[blk, 0]
        wp[:, cb * 4 + 1] = w[blk, 1]
        wp[:, cb * 4 + 2] = w[blk, 2]
        wp[:, cb * 4 + 3] = b[blk]
    return wp


def _pack_diag(w):
    """[128, 3*NBLK*128] bf16: diag(w_k) per (block, tap)."""
    import ml_dtypes

    w = np.asarray(w, dtype=np.float32).reshape(C, K)
    wd = np.zeros((128, 3 * NBLK * 128), np.float32)
    for cb in range(NBLK):
        for k in range(3):
            o = (cb * 3 + k) * 128
            wd[np.arange(128), o + np.arange(128)] = w[cb * 128 : (cb + 1) * 128, k]
    return wd.astype(ml_dtypes.bfloat16)


def _run(inputs, trace=False, **kw):
    import ml_dtypes

    from concourse.bass_utils import run_bass_kernel_spmd

    if "nc" not in _STATE:
        _STATE["nc"] = _build_program()
    nc = _STATE["nc"]

    x = np.asarray(inputs["x"], dtype=np.float32)
    xq = np.zeros((B, C, LP), dtype=ml_dtypes.bfloat16)
    xq[:, :, 1 : L + 1] = x.astype(ml_dtypes.bfloat16)
    wp = _pack_weights(inputs["w"], inputs["b"])
    wd = _pack_diag(inputs["w"])
    in_maps = [
        {"x": xq[c * B_SH : (c + 1) * B_SH], "wpack": wp, "wdiag": wd}
        for c in range(N_CORES)
    ]
    res = run_bass_kernel_spmd(
        nc, in_maps, core_ids=list(range(N_CORES)), trace=trace, **kw
    )
    out = np.concatenate([res.results[c]["out"] for c in range(N_CORES)], axis=0)
    return out.astype(np.float32), res


def kernel(**inputs):
    return _run(inputs)[0]
